# revision 7
# baseline (speedup 1.0000x reference)
"""Trainium2 Bass kernel for nn_AttentionLayer (linear attention, sparse_attention).

Math (per batch element n of B*H*W=2304):
    q = qin @ Wq + bq                (N=80 tokens, C=256 -> 128)
    k = [x|guidance] @ Wk + bk       (S=77 tokens)
    v = x @ Wv + bv
    Q = elu(q)+1, K = elu(k)+1       (8 heads x 16 dim)
    ZD[l,h]   = sum_d Q[l,hd]*Ksum[hd],  Ksum = sum_s K
    Z         = 1/(ZD+eps)
    Qbar[hd]  = sum_l Q[l,hd]*Z[l,h]
    A[h,s]    = sum_d Qbar[hd]*K[s,hd]
    out[hv]   = (1/N) * sum_s A[h,s]*v[s,hv]     (exact refactor of reference)

Feature layout: feature dim (128) on SBUF partitions, tokens on the free axis.
Host supplies pre-transposed bf16 inputs; output is transposed back on host.

v2 engine split (vs v1):
  ACT    : exp+relu of q/k (bias fused), A^T copy out of PSUM
  GPSIMD : fm combine  fm = min(exp,1)+relu  as one scalar_tensor_tensor,
           kbd/abd mask*broadcast products
  DVE    : v copy (+bias), Ksum reduce, Z = approx-recip of ZD (fp32->bf16,
           eps dropped: ZD >> 1e-6 always), group-wide products against the
           expander outputs + segmented tensor_reduce (replaces v1's
           per-element stt+accum chains)
  PE     : projections, ZD/A small matmuls, Z/A expanders (no init matmuls:
           accumulation-free start=True everywhere, garbage PSUM rows are
           never read)
"""

import numpy as np
import ml_dtypes

import concourse.bass as bass
import concourse.bacc as bacc
import concourse.mybir as mybir
import concourse.tile as tile
from concourse.bass_utils import run_bass_kernel_spmd
from concourse.dve_ops import RECIP_APPROX_FAST_CONSTS, RECIPROCAL_APPROX_FAST

F32 = mybir.dt.float32
BF16 = mybir.dt.bfloat16
AF = mybir.ActivationFunctionType
ALU = mybir.AluOpType


NCORES = 8
NH, DH, HID = 8, 16, 128
S, NTOK, C = 77, 80, 256
B, N_, H_, W_ = 4, 80, 24, 24
NTOTAL = B * H_ * W_            # 2304
NLOC = NTOTAL // NCORES         # 288
GRP = 6                         # batch elements per group
NGRP_SUPER = 4                  # groups per supergroup (4 x 8 rows in PSUM bands)
SUPER = GRP * NGRP_SUPER        # 24 n per supergroup


def build_nc(n_loc: int = NLOC) -> bass.Bass:
    assert n_loc % SUPER == 0
    nsuper = n_loc // SUPER

    nc = bacc.Bacc()

    qt = nc.declare_dram_parameter("qt", [2, HID, n_loc * NTOK], BF16, isOutput=False)
    xt = nc.declare_dram_parameter("xt", [HID, n_loc * S], BF16, isOutput=False)
    gt = nc.declare_dram_parameter("gt", [HID, n_loc * S], BF16, isOutput=False)
    CBW = 6 * HID + 2 * GRP * NH + 8   # wq0|wq1|wk0|wk1|wv|e8|m1|mn|f32bits
    cbp = nc.declare_dram_parameter("cb", [HID, CBW], BF16, isOutput=False)
    out = nc.declare_dram_parameter("o", [HID, n_loc], F32, isOutput=True)

    QF = GRP * NTOK   # 480 free elems per group (q side)
    KF = GRP * S      # 462 free elems per group (k/v side)
    QS = SUPER * NTOK  # 1920 per supergroup
    KS = SUPER * S     # 1848

    rc = RECIP_APPROX_FAST_CONSTS

    with tile.TileContext(nc) as tc:
        with (
            tc.tile_pool(name="consts", bufs=1) as consts,
            tc.tile_pool(name="dmain", bufs=2) as dmain,
            tc.tile_pool(name="work", bufs=3) as work,
            tc.tile_pool(name="persist", bufs=6) as persist,
            tc.tile_pool(name="za", bufs=2) as za,
            tc.tile_pool(name="small", bufs=4) as small,
            tc.tile_pool(name="outp", bufs=1) as outp,
            tc.tile_pool(name="pq", bufs=2, space="PSUM") as pqp,
            tc.tile_pool(name="pk", bufs=2, space="PSUM") as pkp,
            tc.tile_pool(name="pv", bufs=1, space="PSUM") as pvp,
            tc.tile_pool(name="pzd", bufs=1, space="PSUM") as pzdp,
            tc.tile_pool(name="pe", bufs=1, space="PSUM") as pep,
            tc.tile_pool(name="pa", bufs=1, space="PSUM") as pap,
        ):
            # ---- constants: one packed blob, one DMA ----
            cb_t = consts.tile([HID, CBW], BF16)
            nc.sync.dma_start(cb_t[:], cbp[:])
            wq0_t = cb_t[:, 0:HID]
            wq1_t = cb_t[:, HID:2 * HID]
            wk0_t = cb_t[:, 2 * HID:3 * HID]
            wk1_t = cb_t[:, 3 * HID:4 * HID]
            wv_t = cb_t[:, 4 * HID:5 * HID]
            e8_t = cb_t[:, 5 * HID:6 * HID]
            m1_t = cb_t[:, 6 * HID:6 * HID + GRP * NH]
            mn_t = cb_t[:, 6 * HID + GRP * NH:6 * HID + 2 * GRP * NH]
            fb_t = cb_t[:, CBW - 8:CBW].bitcast(F32)
            bq_t = fb_t[:, 0:1]
            bk_t = fb_t[:, 1:2]
            bv_t = fb_t[:, 2:3]
            tc.strict_bb_all_engine_barrier()

            # HAM warm-up: ~14 chained matmuls of garbage into the expander
            # bank keep the PE busy >3.4us straight so the clock gate opens to
            # 8/8 before real work (which otherwise never sustains a full
            # activity window and runs at 1.2 GHz throughout). Runs during the
            # first supergroup's input DMA.
            heat = pep.tile([HID, 512], F32, tag="pze")
            for h in range(14):
                nc.tensor.matmul(heat[:], cb_t[:, 0:HID], cb_t[:, 0:512],
                                 start=(h == 0), stop=(h == 13))

            outT = outp.tile([HID, n_loc], F32)

            for sg in range(nsuper):
                # ---- supergroup DMA in ----
                qt_sb = dmain.tile([HID, 2, QS], BF16, tag="qt")
                xt_sb = dmain.tile([HID, KS], BF16, tag="xt")
                gt_sb = dmain.tile([HID, KS], BF16, tag="gt")
                for po in range(2):
                    nc.sync.dma_start(
                        qt_sb[:, po, :], qt[po, :, sg * QS:(sg + 1) * QS]
                    )
                nc.sync.dma_start(xt_sb[:], xt[:, sg * KS:(sg + 1) * KS])
                nc.sync.dma_start(gt_sb[:], gt[:, sg * KS:(sg + 1) * KS])

                # supergroup-lifetime PSUM bands (4 groups x 8 rows each);
                # rows 32g+8..32g+32 are never written and never read.
                pzd = pzdp.tile([HID, 512], F32, tag="pzd")
                pzd = pzd[:, :QF]
                pa = pap.tile([HID, 512], F32, tag="pa")
                pa = pa[:, :KF]

                kfm_g = []
                qfm_g = []
                vsb_g = []
                # ================= front half: proj + fm + ZD =================
                # Projections run in group pairs sharing each stationary load
                # (wq0 g,g+1 then wq1 g,g+1 ...) for longer PE bursts and
                # fewer LDWEIGHTS.
                pq_g = {}
                pk_g = {}
                for g in range(NGRP_SUPER):
                    if g % 2 == 0:
                        pair = (g, g + 1)
                        for gg in pair:
                            pq_t = pqp.tile([HID, 512], F32, tag="pq")
                            pk_t = pkp.tile([HID, 512], F32, tag="pk")
                            pq_g[gg] = pq_t
                            pk_g[gg] = pk_t
                        for w_t, po in ((wq0_t, 0), (wq1_t, 1)):
                            for gg in pair:
                                qs2 = slice(gg * QF, (gg + 1) * QF)
                                nc.tensor.matmul(
                                    pq_g[gg][:, :QF], w_t, qt_sb[:, po, qs2],
                                    start=(po == 0), stop=(po == 1))
                        for w_t, src in ((wk0_t, xt_sb), (wk1_t, gt_sb)):
                            for gg in pair:
                                ks2 = slice(gg * KF, (gg + 1) * KF)
                                nc.tensor.matmul(
                                    pk_g[gg][:, :KF], w_t, src[:, ks2],
                                    start=(w_t is wk0_t), stop=(w_t is wk1_t))
                    qs = slice(g * QF, (g + 1) * QF)
                    ks = slice(g * KF, (g + 1) * KF)
                    pq = pq_g[g][:, :QF]
                    pk = pk_g[g][:, :KF]
                    pv = pvp.tile([HID, 512], F32, tag="pv")
                    pv = pv[:, :KF]
                    nc.tensor.matmul(pv[:], wv_t, xt_sb[:, ks],
                                     start=True, stop=True)

                    # feature map: fm(y) = min(exp(y), 1) + relu(y)
                    eq = work.tile([HID, QF], BF16, tag="eq")
                    rq = work.tile([HID, QF], BF16, tag="rq")
                    qfm = persist.tile([HID, QF], BF16, tag="qfm")
                    ek = work.tile([HID, KF], BF16, tag="ek")
                    rk = work.tile([HID, KF], BF16, tag="rk")
                    kfm = persist.tile([HID, KF], BF16, tag="kfm")
                    vsb = persist.tile([HID, KF], BF16, tag="vsb")
                    nc.scalar.activation(eq[:], pq[:], AF.Exp, bias=bq_t)
                    nc.scalar.activation(rq[:], pq[:], AF.Relu, bias=bq_t)
                    nc.scalar.activation(ek[:], pk[:], AF.Exp, bias=bk_t)
                    nc.scalar.activation(rk[:], pk[:], AF.Relu, bias=bk_t)
                    nc.scalar.activation(vsb[:], pv[:], AF.Identity, bias=bv_t)
                    eqm = work.tile([HID, QF], BF16, tag="eqm")
                    ekm = work.tile([HID, KF], BF16, tag="ekm")
                    nc.vector.tensor_scalar_min(eqm[:], eq[:], 1.0)
                    nc.vector.tensor_scalar_min(ekm[:], ek[:], 1.0)
                    nc.gpsimd.tensor_tensor(qfm[:], eqm[:], rq[:], ALU.add)
                    nc.gpsimd.tensor_tensor(kfm[:], ekm[:], rk[:], ALU.add)

                    # Ksum (128, GRP) then KBD = mask1 * Ksum  (8 cols per n)
                    ksum = small.tile([HID, GRP], F32, tag="ksum")
                    kbd = small.tile([HID, GRP * NH], BF16, tag="kbd")
                    nc.vector.tensor_reduce(
                        ksum[:], kfm[:].rearrange("p (g s) -> p g s", s=S),
                        mybir.AxisListType.X, ALU.add)
                    nc.gpsimd.tensor_tensor(
                        kbd[:].rearrange("p (g h) -> p g h", h=NH),
                        m1_t[:].rearrange("p (g h) -> p g h", h=NH),
                        ksum[:, :, None].to_broadcast((HID, GRP, NH)),
                        ALU.mult)

                    # ZD rows for this group -> packed at partition base 32*g
                    for i in range(GRP):
                        nc.tensor.matmul(
                            pzd[32 * g:32 * g + NH, i * NTOK:(i + 1) * NTOK],
                            kbd[:, i * NH:(i + 1) * NH],
                            qfm[:, i * NTOK:(i + 1) * NTOK],
                            start=True, stop=True, skip_group_check=True,
                            tile_position=(0, 32 * g))
                    kfm_g.append(kfm)
                    qfm_g.append(qfm)
                    vsb_g.append(vsb)

                # ========== supergroup: Z ~= 1/ZD (fast recip, bf16 out) =====
                zpk = za.tile([HID, QF], BF16, tag="zpk")
                nc.vector._custom_dve(
                    RECIPROCAL_APPROX_FAST, out=zpk[:], in0=pzd[:],
                    s0=rc["s0"], s1=rc["s1"], imm2=rc["imm2"])

                # ================= back half =================
                for g in range(NGRP_SUPER):
                    qfm = qfm_g[g]
                    kfm = kfm_g[g]
                    rowg = slice(32 * g, 32 * g + NH)

                    # Zexp (128, 480): one expander matmul per group
                    pze = pep.tile([HID, 512], F32, tag="pze")
                    pze = pze[:, :QF]
                    nc.tensor.matmul(
                        pze[:], e8_t[rowg, :], zpk[rowg, :],
                        start=True, stop=True, tile_position=(32 * g, 0))

                    # Qbar[hd, i] = sum_l qfm * zexp : product + segmented sum
                    prodq = work.tile([HID, QF], BF16, tag="prodq")
                    qbar = small.tile([HID, GRP], F32, tag="qbar")
                    nc.vector.tensor_tensor(prodq[:], qfm[:], pze[:], ALU.mult)
                    nc.vector.tensor_reduce(
                        qbar[:], prodq[:].rearrange("p (g l) -> p g l", l=NTOK),
                        mybir.AxisListType.X, ALU.add)

                    # Abd = maskn * Qbar ; A^T rows packed at base 32*g
                    abd = small.tile([HID, GRP * NH], BF16, tag="abd")
                    nc.gpsimd.tensor_tensor(
                        abd[:].rearrange("p (g h) -> p g h", h=NH),
                        mn_t[:].rearrange("p (g h) -> p g h", h=NH),
                        qbar[:, :, None].to_broadcast((HID, GRP, NH)),
                        ALU.mult)
                    for i in range(GRP):
                        nc.tensor.matmul(
                            pa[rowg, i * S:(i + 1) * S],
                            abd[:, i * NH:(i + 1) * NH],
                            kfm[:, i * S:(i + 1) * S],
                            start=True, stop=True, skip_group_check=True,
                            tile_position=(0, 32 * g))

                # A^T -> SBUF bf16 once per supergroup
                apk = za.tile([HID, KF], BF16, tag="apk")
                nc.scalar.activation(apk[:], pa[:], AF.Copy)

                for g in range(NGRP_SUPER):
                    vsb = vsb_g[g]
                    rowg = slice(32 * g, 32 * g + NH)
                    pae = pep.tile([HID, 512], F32, tag="pze")
                    pae = pae[:, :KF]
                    nc.tensor.matmul(
                        pae[:], e8_t[rowg, :], apk[rowg, :],
                        start=True, stop=True, tile_position=(32 * g, 0))
                    # out^T[:, n] = sum_s vsb * aexp : product + segmented sum
                    prodv = work.tile([HID, KF], BF16, tag="prodv")
                    nc.vector.tensor_tensor(prodv[:], vsb[:], pae[:], ALU.mult)
                    ocol = sg * SUPER + g * GRP
                    nc.vector.tensor_reduce(
                        outT[:, ocol:ocol + GRP],
                        prodv[:].rearrange("p (g s) -> p g s", s=S),
                        mybir.AxisListType.X, ALU.add)

            nc.sync.dma_start(out[:], outT[:])

    nc.finalize()
    return nc


# ---------------- host-side packing ----------------

def make_consts():
    hd = np.arange(HID)
    e8 = (hd[None, :] // DH == (np.arange(HID) % NH)[:, None]).astype(np.float32)
    m1 = np.zeros((HID, GRP * NH), np.float32)
    for i in range(GRP):
        for h in range(NH):
            m1[h * DH:(h + 1) * DH, i * NH + h] = 1.0
    mn = (m1 / float(NTOK)).astype(np.float32)
    return e8, m1, mn


def shard_inputs(query, x, guidance, Wq, bq, Wk, bk, Wv, bv, n_loc=NLOC,
                 ncores=NCORES):
    qin = np.ascontiguousarray(
        query.transpose(0, 2, 3, 1, 4)).reshape(NTOTAL, NTOK, C)
    e8, m1, mn = make_consts()
    bf = ml_dtypes.bfloat16
    wqr = Wq.reshape(2, HID, HID)
    wkr = Wk.reshape(2, HID, HID)
    cb = np.concatenate(
        [wqr[0], wqr[1], wkr[0], wkr[1], Wv, e8, m1, mn], axis=1).astype(bf)
    fb = np.stack(
        [bq, bk, bv, np.zeros(HID, np.float32)], axis=1).astype(np.float32)
    fb_as_bf = np.ascontiguousarray(fb).view(bf)
    cb = np.concatenate([cb, fb_as_bf], axis=1)
    shared = dict(cb=cb)
    in_maps = []
    for i in range(ncores):
        sl = slice(i * n_loc, (i + 1) * n_loc)
        qc = qin[sl].reshape(n_loc * NTOK, C)
        xc = x[sl].reshape(n_loc * S, HID)
        gc = guidance[sl].reshape(n_loc * S, HID)
        m = dict(shared)
        m["qt"] = np.ascontiguousarray(qc.T).reshape(2, HID, n_loc * NTOK).astype(bf)
        m["xt"] = np.ascontiguousarray(xc.T).astype(bf)
        m["gt"] = np.ascontiguousarray(gc.T).astype(bf)
        in_maps.append(m)
    return in_maps


_NC_CACHE = {}


def kernel(**inputs) -> np.ndarray:
    inputs = {k: np.asarray(v, dtype=np.float32) if np.asarray(v).dtype != np.int32
              else np.asarray(v) for k, v in inputs.items()}
    in_maps = shard_inputs(**inputs)
    if NLOC not in _NC_CACHE:
        _NC_CACHE[NLOC] = build_nc(NLOC)
    nc = _NC_CACHE[NLOC]
    res = run_bass_kernel_spmd(nc, in_maps, core_ids=list(range(NCORES)))
    outs = [np.asarray(res.results[i]["o"]).T for i in range(NCORES)]
    full = np.concatenate(outs, axis=0)  # (2304, 128)
    return full.reshape(B, H_, W_, HID).astype(np.float32)


# revision 11
# speedup vs baseline: 1.0099x; 1.0099x over previous
"""Trainium2 Bass kernel for nn_AttentionLayer (linear attention, sparse_attention).

Math (per batch element n of B*H*W=2304):
    q = qin @ Wq + bq                (N=80 tokens, C=256 -> 128)
    k = [x|guidance] @ Wk + bk       (S=77 tokens)
    v = x @ Wv + bv
    Q = elu(q)+1, K = elu(k)+1       (8 heads x 16 dim)
    ZD[l,h]   = sum_d Q[l,hd]*Ksum[hd],  Ksum = sum_s K
    Z         = 1/(ZD+eps)
    Qbar[hd]  = sum_l Q[l,hd]*Z[l,h]
    A[h,s]    = sum_d Qbar[hd]*K[s,hd]
    out[hv]   = (1/N) * sum_s A[h,s]*v[s,hv]     (exact refactor of reference)

Feature layout: feature dim (128) on SBUF partitions, tokens on the free axis.
Host supplies pre-transposed inputs; output is transposed back on host.

v3: fp8(e4m3) inputs + DoubleRow projections. q/k projections contract all
256 input channels in ONE matmul (stationary holds 2 fp8 weights per PE
cell), halving projection PE time and halving input DMA. The PE runs cold
(1.2 GHz) because this dataflow can't sustain a full HAM activity window, so
fewer columns/instructions matter double.

Engine split:
  ACT    : exp+relu of q/k (bias fused), v copy (+bias), A^T copy
  GPSIMD : fm adds (min(exp,1)+relu), kbd/abd mask*broadcast products
  DVE    : min(exp,1), Ksum reduce, Z = approx-recip of ZD (eps dropped:
           ZD >> 1e-6 always), products against expander outputs +
           segmented tensor_reduce
  PE     : projections (fp8), ZD/A small matmuls, Z/A expanders (bf16)
"""

import numpy as np
import ml_dtypes

import concourse.bass as bass
import concourse.bacc as bacc
import concourse.mybir as mybir
import concourse.tile as tile
from concourse.bass_utils import run_bass_kernel_spmd
from concourse.dve_ops import RECIP_APPROX_FAST_CONSTS, RECIPROCAL_APPROX_FAST

F32 = mybir.dt.float32
BF16 = mybir.dt.bfloat16
FP8 = mybir.dt.float8e4
AF = mybir.ActivationFunctionType
ALU = mybir.AluOpType
DR = mybir.MatmulPerfMode.DoubleRow


NCORES = 8
NH, DH, HID = 8, 16, 128
S, NTOK, C = 77, 80, 256
B, N_, H_, W_ = 4, 80, 24, 24
NTOTAL = B * H_ * W_            # 2304
NLOC = NTOTAL // NCORES         # 288
GRP = 6                         # batch elements per group
NGRP_SUPER = 4                  # groups per supergroup (4 x 8 rows in PSUM bands)
SUPER = GRP * NGRP_SUPER        # 24 n per supergroup


def build_nc(n_loc: int = NLOC) -> bass.Bass:
    assert n_loc % SUPER == 0
    nsuper = n_loc // SUPER

    nc = bacc.Bacc()

    qt = nc.declare_dram_parameter("qt", [2, HID, n_loc * NTOK], FP8, isOutput=False)
    xt = nc.declare_dram_parameter("xt", [HID, n_loc * S], BF16, isOutput=False)
    gt = nc.declare_dram_parameter("gt", [HID, n_loc * S], BF16, isOutput=False)
    c8p = nc.declare_dram_parameter("c8", [HID, 2 * HID], FP8, isOutput=False)
    CBW = 4 * HID + 2 * GRP * NH + 8   # wk0|wk1|wv|e8|m1|mn|f32bits
    cbp = nc.declare_dram_parameter("cb", [HID, CBW], BF16, isOutput=False)
    out = nc.declare_dram_parameter("o", [HID, n_loc], F32, isOutput=True)

    QF = GRP * NTOK   # 480 free elems per group (q side)
    KF = GRP * S      # 462 free elems per group (k/v side)
    QS = SUPER * NTOK  # 1920 per supergroup
    KS = SUPER * S     # 1848

    rc = RECIP_APPROX_FAST_CONSTS

    with tile.TileContext(nc) as tc:
        with (
            tc.tile_pool(name="consts", bufs=1) as consts,
            tc.tile_pool(name="dmain", bufs=2) as dmain,
            tc.tile_pool(name="work", bufs=3) as work,
            tc.tile_pool(name="persist", bufs=6) as persist,
            tc.tile_pool(name="za", bufs=2) as za,
            tc.tile_pool(name="small", bufs=4) as small,
            tc.tile_pool(name="outp", bufs=1) as outp,
            tc.tile_pool(name="pq", bufs=2, space="PSUM") as pqp,
            tc.tile_pool(name="pk", bufs=2, space="PSUM") as pkp,
            tc.tile_pool(name="pv", bufs=1, space="PSUM") as pvp,
            tc.tile_pool(name="pzd", bufs=1, space="PSUM") as pzdp,
            tc.tile_pool(name="pe", bufs=1, space="PSUM") as pep,
            tc.tile_pool(name="pa", bufs=1, space="PSUM") as pap,
        ):
            # ---- constants: two packed blobs, two DMAs ----
            cb_t = consts.tile([HID, CBW], BF16)
            nc.sync.dma_start(cb_t[:], cbp[:])
            c8_t = consts.tile([HID, 2 * HID], FP8)
            nc.sync.dma_start(c8_t[:], c8p[:])
            # DoubleRow stationary: [Ki=128, Ko=2, M=128]
            wq_dr = c8_t[:, 0:2 * HID].rearrange("p (ko m) -> p ko m", m=HID)
            wk0_t = cb_t[:, 0:HID]
            wk1_t = cb_t[:, HID:2 * HID]
            wv_t = cb_t[:, 2 * HID:3 * HID]
            e8_t = cb_t[:, 3 * HID:4 * HID]
            m1_t = cb_t[:, 4 * HID:4 * HID + GRP * NH]
            mn_t = cb_t[:, 4 * HID + GRP * NH:4 * HID + 2 * GRP * NH]
            fb_t = cb_t[:, CBW - 8:CBW].bitcast(F32)
            bq_t = fb_t[:, 0:1]
            bk_t = fb_t[:, 1:2]
            bv_t = fb_t[:, 2:3]
            tc.strict_bb_all_engine_barrier()

            # HAM warm-up: chained garbage matmuls keep the PE busy >3.4us so
            # the clock gate opens before real work; overlaps the first input
            # DMA. (Steady state re-throttles regardless; this helps the head.)
            heat = pep.tile([HID, 512], F32, tag="pze")
            for h in range(14):
                nc.tensor.matmul(heat[:], cb_t[:, 0:HID], cb_t[:, 0:512],
                                 start=(h == 0), stop=(h == 13))

            outT = outp.tile([HID, n_loc], F32)

            for sg in range(nsuper):
                # ---- supergroup DMA in (fp8) ----
                qt_sb = dmain.tile([HID, 2, QS], FP8, tag="qt")
                kg_sb = dmain.tile([HID, 2, KS], BF16, tag="kg")
                for po in range(2):
                    nc.sync.dma_start(
                        qt_sb[:, po, :], qt[po, :, sg * QS:(sg + 1) * QS]
                    )
                nc.sync.dma_start(kg_sb[:, 0, :], xt[:, sg * KS:(sg + 1) * KS])
                nc.sync.dma_start(kg_sb[:, 1, :], gt[:, sg * KS:(sg + 1) * KS])

                # supergroup-lifetime PSUM bands (4 groups x 8 rows each);
                # rows 32g+8..32g+32 are never written and never read.
                pzd = pzdp.tile([HID, 512], F32, tag="pzd")
                pzd = pzd[:, :QF]
                pa = pap.tile([HID, 512], F32, tag="pa")
                pa = pa[:, :KF]

                kfm_g = []
                qfm_g = []
                vsb_g = []
                # ================= front half: proj + fm + ZD =================
                for g in range(NGRP_SUPER):
                    qs = slice(g * QF, (g + 1) * QF)
                    ks = slice(g * KF, (g + 1) * KF)

                    pq = pqp.tile([HID, 512], F32, tag="pq")
                    pq = pq[:, :QF]
                    pk = pkp.tile([HID, 512], F32, tag="pk")
                    pk = pk[:, :KF]
                    pv = pvp.tile([HID, 512], F32, tag="pv")
                    pv = pv[:, :KF]
                    nc.tensor.matmul(pq[:], wq_dr, qt_sb[:, :, qs],
                                     start=True, stop=True, perf_mode=DR)
                    nc.tensor.matmul(pk[:], wk0_t, kg_sb[:, 0, ks],
                                     start=True, stop=False)
                    nc.tensor.matmul(pk[:], wk1_t, kg_sb[:, 1, ks],
                                     start=False, stop=True)
                    nc.tensor.matmul(pv[:], wv_t, kg_sb[:, 0, ks],
                                     start=True, stop=True)

                    # feature map: fm(y) = min(exp(y), 1) + relu(y)
                    eq = work.tile([HID, QF], BF16, tag="eq")
                    rq = work.tile([HID, QF], BF16, tag="rq")
                    qfm = persist.tile([HID, QF], BF16, tag="qfm")
                    ek = work.tile([HID, KF], BF16, tag="ek")
                    rk = work.tile([HID, KF], BF16, tag="rk")
                    kfm = persist.tile([HID, KF], BF16, tag="kfm")
                    vsb = persist.tile([HID, KF], BF16, tag="vsb")
                    nc.scalar.activation(eq[:], pq[:], AF.Exp, bias=bq_t)
                    nc.scalar.activation(rq[:], pq[:], AF.Relu, bias=bq_t)
                    nc.scalar.activation(ek[:], pk[:], AF.Exp, bias=bk_t)
                    nc.scalar.activation(rk[:], pk[:], AF.Relu, bias=bk_t)
                    nc.scalar.activation(vsb[:], pv[:], AF.Identity, bias=bv_t)
                    eqm = work.tile([HID, QF], BF16, tag="eqm")
                    ekm = work.tile([HID, KF], BF16, tag="ekm")
                    nc.vector.tensor_scalar_min(eqm[:], eq[:], 1.0)
                    nc.vector.tensor_scalar_min(ekm[:], ek[:], 1.0)
                    nc.gpsimd.tensor_tensor(qfm[:], eqm[:], rq[:], ALU.add)
                    nc.gpsimd.tensor_tensor(kfm[:], ekm[:], rk[:], ALU.add)

                    # Ksum (128, GRP) then KBD = mask1 * Ksum  (8 cols per n)
                    ksum = small.tile([HID, GRP], F32, tag="ksum")
                    kbd = small.tile([HID, GRP * NH], BF16, tag="kbd")
                    nc.vector.tensor_reduce(
                        ksum[:], kfm[:].rearrange("p (g s) -> p g s", s=S),
                        mybir.AxisListType.X, ALU.add)
                    nc.gpsimd.tensor_tensor(
                        kbd[:].rearrange("p (g h) -> p g h", h=NH),
                        m1_t[:].rearrange("p (g h) -> p g h", h=NH),
                        ksum[:, :, None].to_broadcast((HID, GRP, NH)),
                        ALU.mult)

                    # ZD rows for this group -> packed at partition base 32*g
                    for i in range(GRP):
                        nc.tensor.matmul(
                            pzd[32 * g:32 * g + NH, i * NTOK:(i + 1) * NTOK],
                            kbd[:, i * NH:(i + 1) * NH],
                            qfm[:, i * NTOK:(i + 1) * NTOK],
                            start=True, stop=True, skip_group_check=True,
                            tile_position=(0, 32 * g))
                    kfm_g.append(kfm)
                    qfm_g.append(qfm)
                    vsb_g.append(vsb)

                # ========== supergroup: Z ~= 1/ZD (fast recip, bf16 out) =====
                zpk = za.tile([HID, QF], BF16, tag="zpk")
                nc.vector._custom_dve(
                    RECIPROCAL_APPROX_FAST, out=zpk[:], in0=pzd[:],
                    s0=rc["s0"], s1=rc["s1"], imm2=rc["imm2"])

                # ================= back half =================
                for g in range(NGRP_SUPER):
                    qfm = qfm_g[g]
                    kfm = kfm_g[g]
                    rowg = slice(32 * g, 32 * g + NH)

                    # Zexp (128, 480): one expander matmul per group
                    pze = pep.tile([HID, 512], F32, tag="pze")
                    pze = pze[:, :QF]
                    nc.tensor.matmul(
                        pze[:], e8_t[rowg, :], zpk[rowg, :],
                        start=True, stop=True, tile_position=(32 * g, 0))

                    # Qbar[hd, i] = sum_l qfm * zexp : product + segmented sum
                    prodq = work.tile([HID, QF], BF16, tag="prodq")
                    qbar = small.tile([HID, GRP], F32, tag="qbar")
                    nc.vector.tensor_tensor(prodq[:], qfm[:], pze[:], ALU.mult)
                    nc.vector.tensor_reduce(
                        qbar[:], prodq[:].rearrange("p (g l) -> p g l", l=NTOK),
                        mybir.AxisListType.X, ALU.add)

                    # Abd = maskn * Qbar ; A^T rows packed at base 32*g
                    abd = small.tile([HID, GRP * NH], BF16, tag="abd")
                    nc.gpsimd.tensor_tensor(
                        abd[:].rearrange("p (g h) -> p g h", h=NH),
                        mn_t[:].rearrange("p (g h) -> p g h", h=NH),
                        qbar[:, :, None].to_broadcast((HID, GRP, NH)),
                        ALU.mult)
                    for i in range(GRP):
                        nc.tensor.matmul(
                            pa[rowg, i * S:(i + 1) * S],
                            abd[:, i * NH:(i + 1) * NH],
                            kfm[:, i * S:(i + 1) * S],
                            start=True, stop=True, skip_group_check=True,
                            tile_position=(0, 32 * g))

                # A^T -> SBUF bf16 once per supergroup
                apk = za.tile([HID, KF], BF16, tag="apk")
                nc.scalar.activation(apk[:], pa[:], AF.Copy)

                for g in range(NGRP_SUPER):
                    vsb = vsb_g[g]
                    rowg = slice(32 * g, 32 * g + NH)
                    pae = pep.tile([HID, 512], F32, tag="pze")
                    pae = pae[:, :KF]
                    nc.tensor.matmul(
                        pae[:], e8_t[rowg, :], apk[rowg, :],
                        start=True, stop=True, tile_position=(32 * g, 0))
                    # out^T[:, n] = sum_s vsb * aexp : product + segmented sum
                    prodv = work.tile([HID, KF], BF16, tag="prodv")
                    nc.vector.tensor_tensor(prodv[:], vsb[:], pae[:], ALU.mult)
                    ocol = sg * SUPER + g * GRP
                    nc.vector.tensor_reduce(
                        outT[:, ocol:ocol + GRP],
                        prodv[:].rearrange("p (g s) -> p g s", s=S),
                        mybir.AxisListType.X, ALU.add)

            nc.sync.dma_start(out[:], outT[:])

    nc.finalize()
    return nc


# ---------------- host-side packing ----------------

def make_consts():
    hd = np.arange(HID)
    e8 = (hd[None, :] // DH == (np.arange(HID) % NH)[:, None]).astype(np.float32)
    m1 = np.zeros((HID, GRP * NH), np.float32)
    for i in range(GRP):
        for h in range(NH):
            m1[h * DH:(h + 1) * DH, i * NH + h] = 1.0
    mn = (m1 / float(NTOK)).astype(np.float32)
    return e8, m1, mn


def shard_inputs(query, x, guidance, Wq, bq, Wk, bk, Wv, bv, n_loc=NLOC,
                 ncores=NCORES):
    qin = np.ascontiguousarray(
        query.transpose(0, 2, 3, 1, 4)).reshape(NTOTAL, NTOK, C)
    e8, m1, mn = make_consts()
    bf = ml_dtypes.bfloat16
    f8 = ml_dtypes.float8_e4m3
    wqr = Wq.reshape(2, HID, HID)
    wkr = Wk.reshape(2, HID, HID)
    # DoubleRow stationaries [Ki, Ko, M] flattened to [Ki, Ko*M]
    wq_dr = np.stack([wqr[0], wqr[1]], axis=1).reshape(HID, 2 * HID)
    c8 = wq_dr.astype(f8)
    cb = np.concatenate([wkr[0], wkr[1], Wv, e8, m1, mn], axis=1).astype(bf)
    fb = np.stack(
        [bq, bk, bv, np.zeros(HID, np.float32)], axis=1).astype(np.float32)
    fb_as_bf = np.ascontiguousarray(fb).view(bf)
    cb = np.concatenate([cb, fb_as_bf], axis=1)
    shared = dict(cb=cb, c8=c8)
    in_maps = []
    for i in range(ncores):
        sl = slice(i * n_loc, (i + 1) * n_loc)
        qc = qin[sl].reshape(n_loc * NTOK, C)
        xc = x[sl].reshape(n_loc * S, HID)
        gc = guidance[sl].reshape(n_loc * S, HID)
        m = dict(shared)
        m["qt"] = np.ascontiguousarray(qc.T).reshape(2, HID, n_loc * NTOK).astype(f8)
        m["xt"] = np.ascontiguousarray(xc.T).astype(bf)
        m["gt"] = np.ascontiguousarray(gc.T).astype(bf)
        in_maps.append(m)
    return in_maps


_NC_CACHE = {}


def kernel(**inputs) -> np.ndarray:
    inputs = {k: np.asarray(v, dtype=np.float32) if np.asarray(v).dtype != np.int32
              else np.asarray(v) for k, v in inputs.items()}
    in_maps = shard_inputs(**inputs)
    if NLOC not in _NC_CACHE:
        _NC_CACHE[NLOC] = build_nc(NLOC)
    nc = _NC_CACHE[NLOC]
    res = run_bass_kernel_spmd(nc, in_maps, core_ids=list(range(NCORES)))
    outs = [np.asarray(res.results[i]["o"]).T for i in range(NCORES)]
    full = np.concatenate(outs, axis=0)  # (2304, 128)
    return full.reshape(B, H_, W_, HID).astype(np.float32)


# revision 12
# speedup vs baseline: 1.0449x; 1.0347x over previous
"""Trainium2 Bass kernel for nn_AttentionLayer (linear attention, sparse_attention).

Math (per batch element n of B*H*W=2304):
    q = qin @ Wq + bq                (N=80 tokens, C=256 -> 128)
    k = [x|guidance] @ Wk + bk       (S=77 tokens)
    v = x @ Wv + bv
    Q = elu(q)+1, K = elu(k)+1       (8 heads x 16 dim)
    ZD[l,h]   = sum_d Q[l,hd]*Ksum[hd],  Ksum = sum_s K
    Z         = 1/(ZD+eps)
    Qbar[hd]  = sum_l Q[l,hd]*Z[l,h]
    A[h,s]    = sum_d Qbar[hd]*K[s,hd]
    out[hv]   = (1/N) * sum_s A[h,s]*v[s,hv]     (exact refactor of reference)

Feature layout: feature dim (128) on SBUF partitions, tokens on the free axis.
Host supplies pre-transposed inputs; output is transposed back on host.

v3: fp8(e4m3) inputs + DoubleRow projections. q/k projections contract all
256 input channels in ONE matmul (stationary holds 2 fp8 weights per PE
cell), halving projection PE time and halving input DMA. The PE runs cold
(1.2 GHz) because this dataflow can't sustain a full HAM activity window, so
fewer columns/instructions matter double.

Engine split:
  ACT    : exp+relu of q/k (bias fused), v copy (+bias), A^T copy
  GPSIMD : fm adds (min(exp,1)+relu), kbd/abd mask*broadcast products
  DVE    : min(exp,1), Ksum reduce, Z = approx-recip of ZD (eps dropped:
           ZD >> 1e-6 always), products against expander outputs +
           segmented tensor_reduce
  PE     : projections (fp8), ZD/A small matmuls, Z/A expanders (bf16)
"""

import numpy as np
import ml_dtypes

import concourse.bass as bass
import concourse.bacc as bacc
import concourse.mybir as mybir
import concourse.tile as tile
from concourse.bass_utils import run_bass_kernel_spmd
from concourse.dve_ops import RECIP_APPROX_FAST_CONSTS, RECIPROCAL_APPROX_FAST

F32 = mybir.dt.float32
BF16 = mybir.dt.bfloat16
FP8 = mybir.dt.float8e4
AF = mybir.ActivationFunctionType
ALU = mybir.AluOpType
DR = mybir.MatmulPerfMode.DoubleRow


NCORES = 8
NH, DH, HID = 8, 16, 128
S, NTOK, C = 77, 80, 256
B, N_, H_, W_ = 4, 80, 24, 24
NTOTAL = B * H_ * W_            # 2304
NLOC = NTOTAL // NCORES         # 288
GRP = 6                         # batch elements per group
NGRP_SUPER = 4                  # groups per supergroup (4 x 8 rows in PSUM bands)
SUPER = GRP * NGRP_SUPER        # 24 n per supergroup


def build_nc(n_loc: int = NLOC) -> bass.Bass:
    assert n_loc % SUPER == 0
    nsuper = n_loc // SUPER

    nc = bacc.Bacc()

    qt = nc.declare_dram_parameter("qt", [2, HID, n_loc * NTOK], FP8, isOutput=False)
    xt = nc.declare_dram_parameter("xt", [HID, n_loc * S], BF16, isOutput=False)
    gt = nc.declare_dram_parameter("gt", [HID, n_loc * S], BF16, isOutput=False)
    c8p = nc.declare_dram_parameter("c8", [HID, 2 * HID], FP8, isOutput=False)
    CBW = 4 * HID + 2 * GRP * NH + 8   # wk0|wk1|wv|e8|m1|mn|f32bits
    cbp = nc.declare_dram_parameter("cb", [HID, CBW], BF16, isOutput=False)
    out = nc.declare_dram_parameter("o", [HID, n_loc], F32, isOutput=True)

    QF = GRP * NTOK   # 480 free elems per group (q side)
    KF = GRP * S      # 462 free elems per group (k/v side)
    QS = SUPER * NTOK  # 1920 per supergroup
    KS = SUPER * S     # 1848

    rc = RECIP_APPROX_FAST_CONSTS

    with tile.TileContext(nc) as tc:
        with (
            tc.tile_pool(name="consts", bufs=1) as consts,
            tc.tile_pool(name="dmain", bufs=2) as dmain,
            tc.tile_pool(name="work", bufs=4) as work,
            tc.tile_pool(name="persist", bufs=9) as persist,
            tc.tile_pool(name="za", bufs=3) as za,
            tc.tile_pool(name="small", bufs=6) as small,
            tc.tile_pool(name="outp", bufs=1) as outp,
            tc.tile_pool(name="pq", bufs=2, space="PSUM") as pqp,
            tc.tile_pool(name="pk", bufs=2, space="PSUM") as pkp,
            tc.tile_pool(name="pv", bufs=1, space="PSUM") as pvp,
            tc.tile_pool(name="pzd", bufs=1, space="PSUM") as pzdp,
            tc.tile_pool(name="pe", bufs=1, space="PSUM") as pep,
            tc.tile_pool(name="pa", bufs=1, space="PSUM") as pap,
        ):
            # ---- constants: two packed blobs, two DMAs ----
            cb_t = consts.tile([HID, CBW], BF16)
            nc.sync.dma_start(cb_t[:], cbp[:])
            c8_t = consts.tile([HID, 2 * HID], FP8)
            nc.sync.dma_start(c8_t[:], c8p[:])
            # DoubleRow stationary: [Ki=128, Ko=2, M=128]
            wq_dr = c8_t[:, 0:2 * HID].rearrange("p (ko m) -> p ko m", m=HID)
            wk0_t = cb_t[:, 0:HID]
            wk1_t = cb_t[:, HID:2 * HID]
            wv_t = cb_t[:, 2 * HID:3 * HID]
            e8_t = cb_t[:, 3 * HID:4 * HID]
            m1_t = cb_t[:, 4 * HID:4 * HID + GRP * NH]
            mn_t = cb_t[:, 4 * HID + GRP * NH:4 * HID + 2 * GRP * NH]
            fb_t = cb_t[:, CBW - 8:CBW].bitcast(F32)
            bq_t = fb_t[:, 0:1]
            bk_t = fb_t[:, 1:2]
            bv_t = fb_t[:, 2:3]
            tc.strict_bb_all_engine_barrier()

            # HAM warm-up: chained garbage matmuls keep the PE busy >3.4us so
            # the clock gate opens before real work; overlaps the first input
            # DMA. (Steady state re-throttles regardless; this helps the head.)
            heat = pep.tile([HID, 512], F32, tag="pze")
            for h in range(14):
                nc.tensor.matmul(heat[:], cb_t[:, 0:HID], cb_t[:, 0:512],
                                 start=(h == 0), stop=(h == 13))

            outT = outp.tile([HID, n_loc], F32)

            for sg in range(nsuper):
                # ---- supergroup DMA in (fp8) ----
                qt_sb = dmain.tile([HID, 2, QS], FP8, tag="qt")
                kg_sb = dmain.tile([HID, 2, KS], BF16, tag="kg")
                for po in range(2):
                    nc.sync.dma_start(
                        qt_sb[:, po, :], qt[po, :, sg * QS:(sg + 1) * QS]
                    )
                nc.sync.dma_start(kg_sb[:, 0, :], xt[:, sg * KS:(sg + 1) * KS])
                nc.sync.dma_start(kg_sb[:, 1, :], gt[:, sg * KS:(sg + 1) * KS])

                # supergroup-lifetime PSUM bands (4 groups x 8 rows each);
                # rows 32g+8..32g+32 are never written and never read.
                pzd = pzdp.tile([HID, 512], F32, tag="pzd")
                pzd = pzd[:, :QF]
                pa = pap.tile([HID, 512], F32, tag="pa")
                pa = pa[:, :KF]

                kfm_g = []
                qfm_g = []
                vsb_g = []
                # ================= front half: proj + fm + ZD =================
                for g in range(NGRP_SUPER):
                    qs = slice(g * QF, (g + 1) * QF)
                    ks = slice(g * KF, (g + 1) * KF)

                    pq = pqp.tile([HID, 512], F32, tag="pq")
                    pq = pq[:, :QF]
                    pk = pkp.tile([HID, 512], F32, tag="pk")
                    pk = pk[:, :KF]
                    pv = pvp.tile([HID, 512], F32, tag="pv")
                    pv = pv[:, :KF]
                    nc.tensor.matmul(pq[:], wq_dr, qt_sb[:, :, qs],
                                     start=True, stop=True, perf_mode=DR)
                    nc.tensor.matmul(pk[:], wk0_t, kg_sb[:, 0, ks],
                                     start=True, stop=False)
                    nc.tensor.matmul(pk[:], wk1_t, kg_sb[:, 1, ks],
                                     start=False, stop=True)
                    nc.tensor.matmul(pv[:], wv_t, kg_sb[:, 0, ks],
                                     start=True, stop=True)

                    # feature map: fm(y) = min(exp(y), 1) + relu(y)
                    eq = work.tile([HID, QF], BF16, tag="eq")
                    rq = work.tile([HID, QF], BF16, tag="rq")
                    qfm = persist.tile([HID, QF], BF16, tag="qfm")
                    ek = work.tile([HID, KF], BF16, tag="ek")
                    rk = work.tile([HID, KF], BF16, tag="rk")
                    kfm = persist.tile([HID, KF], BF16, tag="kfm")
                    vsb = persist.tile([HID, KF], BF16, tag="vsb")
                    nc.scalar.activation(eq[:], pq[:], AF.Exp, bias=bq_t)
                    nc.scalar.activation(rq[:], pq[:], AF.Relu, bias=bq_t)
                    nc.scalar.activation(ek[:], pk[:], AF.Exp, bias=bk_t)
                    nc.scalar.activation(rk[:], pk[:], AF.Relu, bias=bk_t)
                    nc.scalar.activation(vsb[:], pv[:], AF.Identity, bias=bv_t)
                    eqm = work.tile([HID, QF], BF16, tag="eqm")
                    ekm = work.tile([HID, KF], BF16, tag="ekm")
                    nc.vector.tensor_scalar_min(eqm[:], eq[:], 1.0)
                    nc.vector.tensor_scalar_min(ekm[:], ek[:], 1.0)
                    nc.gpsimd.tensor_tensor(qfm[:], eqm[:], rq[:], ALU.add)
                    nc.gpsimd.tensor_tensor(kfm[:], ekm[:], rk[:], ALU.add)

                    # Ksum (128, GRP) then KBD = mask1 * Ksum  (8 cols per n)
                    ksum = small.tile([HID, GRP], F32, tag="ksum")
                    kbd = small.tile([HID, GRP * NH], BF16, tag="kbd")
                    nc.vector.tensor_reduce(
                        ksum[:], kfm[:].rearrange("p (g s) -> p g s", s=S),
                        mybir.AxisListType.X, ALU.add)
                    nc.gpsimd.tensor_tensor(
                        kbd[:].rearrange("p (g h) -> p g h", h=NH),
                        m1_t[:].rearrange("p (g h) -> p g h", h=NH),
                        ksum[:, :, None].to_broadcast((HID, GRP, NH)),
                        ALU.mult)

                    # ZD rows for this group -> packed at partition base 32*g
                    for i in range(GRP):
                        nc.tensor.matmul(
                            pzd[32 * g:32 * g + NH, i * NTOK:(i + 1) * NTOK],
                            kbd[:, i * NH:(i + 1) * NH],
                            qfm[:, i * NTOK:(i + 1) * NTOK],
                            start=True, stop=True, skip_group_check=True,
                            tile_position=(0, 32 * g))
                    kfm_g.append(kfm)
                    qfm_g.append(qfm)
                    vsb_g.append(vsb)

                # ========== supergroup: Z ~= 1/ZD (fast recip, bf16 out) =====
                zpk = za.tile([HID, QF], BF16, tag="zpk")
                nc.vector._custom_dve(
                    RECIPROCAL_APPROX_FAST, out=zpk[:], in0=pzd[:],
                    s0=rc["s0"], s1=rc["s1"], imm2=rc["imm2"])

                # ================= back half =================
                for g in range(NGRP_SUPER):
                    qfm = qfm_g[g]
                    kfm = kfm_g[g]
                    rowg = slice(32 * g, 32 * g + NH)

                    # Zexp (128, 480): one expander matmul per group
                    pze = pep.tile([HID, 512], F32, tag="pze")
                    pze = pze[:, :QF]
                    nc.tensor.matmul(
                        pze[:], e8_t[rowg, :], zpk[rowg, :],
                        start=True, stop=True, tile_position=(32 * g, 0))

                    # Qbar[hd, i] = sum_l qfm * zexp : product + segmented sum
                    prodq = work.tile([HID, QF], BF16, tag="prodq")
                    qbar = small.tile([HID, GRP], F32, tag="qbar")
                    nc.vector.tensor_tensor(prodq[:], qfm[:], pze[:], ALU.mult)
                    nc.vector.tensor_reduce(
                        qbar[:], prodq[:].rearrange("p (g l) -> p g l", l=NTOK),
                        mybir.AxisListType.X, ALU.add)

                    # Abd = maskn * Qbar ; A^T rows packed at base 32*g
                    abd = small.tile([HID, GRP * NH], BF16, tag="abd")
                    nc.gpsimd.tensor_tensor(
                        abd[:].rearrange("p (g h) -> p g h", h=NH),
                        mn_t[:].rearrange("p (g h) -> p g h", h=NH),
                        qbar[:, :, None].to_broadcast((HID, GRP, NH)),
                        ALU.mult)
                    for i in range(GRP):
                        nc.tensor.matmul(
                            pa[rowg, i * S:(i + 1) * S],
                            abd[:, i * NH:(i + 1) * NH],
                            kfm[:, i * S:(i + 1) * S],
                            start=True, stop=True, skip_group_check=True,
                            tile_position=(0, 32 * g))

                # A^T -> SBUF bf16 once per supergroup
                apk = za.tile([HID, KF], BF16, tag="apk")
                nc.scalar.activation(apk[:], pa[:], AF.Copy)

                for g in range(NGRP_SUPER):
                    vsb = vsb_g[g]
                    rowg = slice(32 * g, 32 * g + NH)
                    pae = pep.tile([HID, 512], F32, tag="pze")
                    pae = pae[:, :KF]
                    nc.tensor.matmul(
                        pae[:], e8_t[rowg, :], apk[rowg, :],
                        start=True, stop=True, tile_position=(32 * g, 0))
                    # out^T[:, n] = sum_s vsb * aexp : product + segmented sum
                    prodv = work.tile([HID, KF], BF16, tag="prodv")
                    nc.vector.tensor_tensor(prodv[:], vsb[:], pae[:], ALU.mult)
                    ocol = sg * SUPER + g * GRP
                    nc.vector.tensor_reduce(
                        outT[:, ocol:ocol + GRP],
                        prodv[:].rearrange("p (g s) -> p g s", s=S),
                        mybir.AxisListType.X, ALU.add)

            nc.sync.dma_start(out[:], outT[:])

    nc.finalize()
    return nc


# ---------------- host-side packing ----------------

def make_consts():
    hd = np.arange(HID)
    e8 = (hd[None, :] // DH == (np.arange(HID) % NH)[:, None]).astype(np.float32)
    m1 = np.zeros((HID, GRP * NH), np.float32)
    for i in range(GRP):
        for h in range(NH):
            m1[h * DH:(h + 1) * DH, i * NH + h] = 1.0
    mn = (m1 / float(NTOK)).astype(np.float32)
    return e8, m1, mn


def shard_inputs(query, x, guidance, Wq, bq, Wk, bk, Wv, bv, n_loc=NLOC,
                 ncores=NCORES):
    qin = np.ascontiguousarray(
        query.transpose(0, 2, 3, 1, 4)).reshape(NTOTAL, NTOK, C)
    e8, m1, mn = make_consts()
    bf = ml_dtypes.bfloat16
    f8 = ml_dtypes.float8_e4m3
    wqr = Wq.reshape(2, HID, HID)
    wkr = Wk.reshape(2, HID, HID)
    # DoubleRow stationaries [Ki, Ko, M] flattened to [Ki, Ko*M]
    wq_dr = np.stack([wqr[0], wqr[1]], axis=1).reshape(HID, 2 * HID)
    c8 = wq_dr.astype(f8)
    cb = np.concatenate([wkr[0], wkr[1], Wv, e8, m1, mn], axis=1).astype(bf)
    fb = np.stack(
        [bq, bk, bv, np.zeros(HID, np.float32)], axis=1).astype(np.float32)
    fb_as_bf = np.ascontiguousarray(fb).view(bf)
    cb = np.concatenate([cb, fb_as_bf], axis=1)
    shared = dict(cb=cb, c8=c8)
    in_maps = []
    for i in range(ncores):
        sl = slice(i * n_loc, (i + 1) * n_loc)
        qc = qin[sl].reshape(n_loc * NTOK, C)
        xc = x[sl].reshape(n_loc * S, HID)
        gc = guidance[sl].reshape(n_loc * S, HID)
        m = dict(shared)
        m["qt"] = np.ascontiguousarray(qc.T).reshape(2, HID, n_loc * NTOK).astype(f8)
        m["xt"] = np.ascontiguousarray(xc.T).astype(bf)
        m["gt"] = np.ascontiguousarray(gc.T).astype(bf)
        in_maps.append(m)
    return in_maps


_NC_CACHE = {}


def kernel(**inputs) -> np.ndarray:
    inputs = {k: np.asarray(v, dtype=np.float32) if np.asarray(v).dtype != np.int32
              else np.asarray(v) for k, v in inputs.items()}
    in_maps = shard_inputs(**inputs)
    if NLOC not in _NC_CACHE:
        _NC_CACHE[NLOC] = build_nc(NLOC)
    nc = _NC_CACHE[NLOC]
    res = run_bass_kernel_spmd(nc, in_maps, core_ids=list(range(NCORES)))
    outs = [np.asarray(res.results[i]["o"]).T for i in range(NCORES)]
    full = np.concatenate(outs, axis=0)  # (2304, 128)
    return full.reshape(B, H_, W_, HID).astype(np.float32)


# revision 13
# speedup vs baseline: 1.1547x; 1.1051x over previous
"""Trainium2 Bass kernel for nn_AttentionLayer (linear attention, sparse_attention).

Math (per batch element n of B*H*W=2304):
    q = qin @ Wq + bq                (N=80 tokens, C=256 -> 128)
    k = [x|guidance] @ Wk + bk       (S=77 tokens)
    v = x @ Wv + bv
    Q = elu(q)+1, K = elu(k)+1       (8 heads x 16 dim)
    ZD[l,h]   = sum_d Q[l,hd]*Ksum[hd],  Ksum = sum_s K
    Z         = 1/(ZD+eps)
    Qbar[hd]  = sum_l Q[l,hd]*Z[l,h]
    A[h,s]    = sum_d Qbar[hd]*K[s,hd]
    out[hv]   = (1/N) * sum_s A[h,s]*v[s,hv]     (exact refactor of reference)

Feature layout: feature dim (128) on SBUF partitions, tokens on the free axis.
Host supplies pre-transposed inputs; output is transposed back on host.

v3: fp8(e4m3) inputs + DoubleRow projections. q/k projections contract all
256 input channels in ONE matmul (stationary holds 2 fp8 weights per PE
cell), halving projection PE time and halving input DMA. The PE runs cold
(1.2 GHz) because this dataflow can't sustain a full HAM activity window, so
fewer columns/instructions matter double.

Engine split:
  ACT    : exp+relu of q/k (bias fused), v copy (+bias), A^T copy
  GPSIMD : fm adds (min(exp,1)+relu), kbd/abd mask*broadcast products
  DVE    : min(exp,1), Ksum reduce, Z = approx-recip of ZD (eps dropped:
           ZD >> 1e-6 always), products against expander outputs +
           segmented tensor_reduce
  PE     : projections (fp8), ZD/A small matmuls, Z/A expanders (bf16)
"""

import numpy as np
import ml_dtypes

import concourse.bass as bass
import concourse.bacc as bacc
import concourse.mybir as mybir
import concourse.tile as tile
from concourse.bass_utils import run_bass_kernel_spmd
from concourse.dve_ops import RECIP_APPROX_FAST_CONSTS, RECIPROCAL_APPROX_FAST

F32 = mybir.dt.float32
BF16 = mybir.dt.bfloat16
FP8 = mybir.dt.float8e4
AF = mybir.ActivationFunctionType
ALU = mybir.AluOpType
DR = mybir.MatmulPerfMode.DoubleRow


NCORES = 8
NH, DH, HID = 8, 16, 128
S, NTOK, C = 77, 80, 256
B, N_, H_, W_ = 4, 80, 24, 24
NTOTAL = B * H_ * W_            # 2304
NLOC = NTOTAL // NCORES         # 288
GRP = 6                         # batch elements per group
NGRP_SUPER = 4                  # groups per supergroup (4 x 8 rows in PSUM bands)
SUPER = GRP * NGRP_SUPER        # 24 n per supergroup


def build_nc(n_loc: int = NLOC) -> bass.Bass:
    assert n_loc % SUPER == 0
    nsuper = n_loc // SUPER

    nc = bacc.Bacc()

    qt = nc.declare_dram_parameter("qt", [2, HID, n_loc * NTOK], FP8, isOutput=False)
    xt = nc.declare_dram_parameter("xt", [HID, n_loc * S], BF16, isOutput=False)
    gt = nc.declare_dram_parameter("gt", [HID, n_loc * S], BF16, isOutput=False)
    c8p = nc.declare_dram_parameter("c8", [HID, 2 * HID], FP8, isOutput=False)
    CBW = 4 * HID + 2 * GRP * NH + 8   # wk0|wk1|wv|e8|m1|mn|f32bits
    cbp = nc.declare_dram_parameter("cb", [HID, CBW], BF16, isOutput=False)
    out = nc.declare_dram_parameter("o", [HID, n_loc], F32, isOutput=True)

    QF = GRP * NTOK   # 480 free elems per group (q side)
    KF = GRP * S      # 462 free elems per group (k/v side)
    QS = SUPER * NTOK  # 1920 per supergroup
    KS = SUPER * S     # 1848

    rc = RECIP_APPROX_FAST_CONSTS

    with tile.TileContext(nc) as tc:
        with (
            tc.tile_pool(name="consts", bufs=1) as consts,
            tc.tile_pool(name="dmain", bufs=2) as dmain,
            tc.tile_pool(name="work", bufs=4) as work,
            tc.tile_pool(name="persist", bufs=9) as persist,
            tc.tile_pool(name="za", bufs=3) as za,
            tc.tile_pool(name="small", bufs=6) as small,
            tc.tile_pool(name="outp", bufs=1) as outp,
            tc.tile_pool(name="pq", bufs=2, space="PSUM") as pqp,
            tc.tile_pool(name="pk", bufs=1, space="PSUM") as pkp,
            tc.tile_pool(name="pv", bufs=1, space="PSUM") as pvp,
            tc.tile_pool(name="pzd", bufs=1, space="PSUM") as pzdp,
            tc.tile_pool(name="pe", bufs=2, space="PSUM") as pep,
            tc.tile_pool(name="pa", bufs=1, space="PSUM") as pap,
        ):
            # ---- constants: two packed blobs, two DMAs ----
            cb_t = consts.tile([HID, CBW], BF16)
            nc.sync.dma_start(cb_t[:], cbp[:])
            c8_t = consts.tile([HID, 2 * HID], FP8)
            nc.sync.dma_start(c8_t[:], c8p[:])
            # DoubleRow stationary: [Ki=128, Ko=2, M=128]
            wq_dr = c8_t[:, 0:2 * HID].rearrange("p (ko m) -> p ko m", m=HID)
            wk0_t = cb_t[:, 0:HID]
            wk1_t = cb_t[:, HID:2 * HID]
            wv_t = cb_t[:, 2 * HID:3 * HID]
            e8_t = cb_t[:, 3 * HID:4 * HID]
            m1_t = cb_t[:, 4 * HID:4 * HID + GRP * NH]
            mn_t = cb_t[:, 4 * HID + GRP * NH:4 * HID + 2 * GRP * NH]
            fb_t = cb_t[:, CBW - 8:CBW].bitcast(F32)
            bq_t = fb_t[:, 0:1]
            bk_t = fb_t[:, 1:2]
            bv_t = fb_t[:, 2:3]
            tc.strict_bb_all_engine_barrier()

            # HAM warm-up: chained garbage matmuls keep the PE busy >3.4us so
            # the clock gate opens before real work; overlaps the first input
            # DMA. (Steady state re-throttles regardless; this helps the head.)
            heat = pep.tile([HID, 512], F32, tag="pze")
            for h in range(14):
                nc.tensor.matmul(heat[:], cb_t[:, 0:HID], cb_t[:, 0:512],
                                 start=(h == 0), stop=(h == 13))

            outT = outp.tile([HID, n_loc], F32)

            for sg in range(nsuper):
                # ---- supergroup DMA in (fp8) ----
                qt_sb = dmain.tile([HID, 2, QS], FP8, tag="qt")
                kg_sb = dmain.tile([HID, 2, KS], BF16, tag="kg")
                for po in range(2):
                    nc.sync.dma_start(
                        qt_sb[:, po, :], qt[po, :, sg * QS:(sg + 1) * QS]
                    )
                nc.sync.dma_start(kg_sb[:, 0, :], xt[:, sg * KS:(sg + 1) * KS])
                nc.sync.dma_start(kg_sb[:, 1, :], gt[:, sg * KS:(sg + 1) * KS])

                # supergroup-lifetime PSUM bands (4 groups x 8 rows each);
                # rows 32g+8..32g+32 are never written and never read.
                pzd = pzdp.tile([HID, 512], F32, tag="pzd")
                pzd = pzd[:, :QF]
                pa = pap.tile([HID, 512], F32, tag="pa")
                pa = pa[:, :KF]

                kfm_g = []
                qfm_g = []
                vsb_g = []
                # ================= front half: proj + fm + ZD =================
                for g in range(NGRP_SUPER):
                    qs = slice(g * QF, (g + 1) * QF)
                    ks = slice(g * KF, (g + 1) * KF)

                    pq = pqp.tile([HID, 512], F32, tag="pq")
                    pq = pq[:, :QF]
                    pk = pkp.tile([HID, 512], F32, tag="pk")
                    pk = pk[:, :KF]
                    pv = pvp.tile([HID, 512], F32, tag="pv")
                    pv = pv[:, :KF]
                    nc.tensor.matmul(pq[:], wq_dr, qt_sb[:, :, qs],
                                     start=True, stop=True, perf_mode=DR)
                    nc.tensor.matmul(pk[:], wk0_t, kg_sb[:, 0, ks],
                                     start=True, stop=False)
                    nc.tensor.matmul(pk[:], wk1_t, kg_sb[:, 1, ks],
                                     start=False, stop=True)
                    nc.tensor.matmul(pv[:], wv_t, kg_sb[:, 0, ks],
                                     start=True, stop=True)

                    # feature map: fm(y) = min(exp(y), 1) + relu(y)
                    eq = work.tile([HID, QF], BF16, tag="eq")
                    rq = work.tile([HID, QF], BF16, tag="rq")
                    qfm = persist.tile([HID, QF], BF16, tag="qfm")
                    ek = work.tile([HID, KF], BF16, tag="ek")
                    rk = work.tile([HID, KF], BF16, tag="rk")
                    kfm = persist.tile([HID, KF], BF16, tag="kfm")
                    vsb = persist.tile([HID, KF], BF16, tag="vsb")
                    nc.scalar.activation(eq[:], pq[:], AF.Exp, bias=bq_t)
                    nc.scalar.activation(rq[:], pq[:], AF.Relu, bias=bq_t)
                    nc.scalar.activation(ek[:], pk[:], AF.Exp, bias=bk_t)
                    nc.scalar.activation(rk[:], pk[:], AF.Relu, bias=bk_t)
                    nc.scalar.activation(vsb[:], pv[:], AF.Identity, bias=bv_t)
                    eqm = work.tile([HID, QF], BF16, tag="eqm")
                    ekm = work.tile([HID, KF], BF16, tag="ekm")
                    nc.vector.tensor_scalar_min(eqm[:], eq[:], 1.0)
                    nc.vector.tensor_scalar_min(ekm[:], ek[:], 1.0)
                    nc.gpsimd.tensor_tensor(qfm[:], eqm[:], rq[:], ALU.add)
                    nc.gpsimd.tensor_tensor(kfm[:], ekm[:], rk[:], ALU.add)

                    # Ksum (128, GRP) then KBD = mask1 * Ksum  (8 cols per n)
                    ksum = small.tile([HID, GRP], F32, tag="ksum")
                    kbd = small.tile([HID, GRP * NH], BF16, tag="kbd")
                    nc.vector.tensor_reduce(
                        ksum[:], kfm[:].rearrange("p (g s) -> p g s", s=S),
                        mybir.AxisListType.X, ALU.add)
                    nc.gpsimd.tensor_tensor(
                        kbd[:].rearrange("p (g h) -> p g h", h=NH),
                        m1_t[:].rearrange("p (g h) -> p g h", h=NH),
                        ksum[:, :, None].to_broadcast((HID, GRP, NH)),
                        ALU.mult)

                    # ZD rows for this group -> packed at partition base 32*g
                    for i in range(GRP):
                        nc.tensor.matmul(
                            pzd[32 * g:32 * g + NH, i * NTOK:(i + 1) * NTOK],
                            kbd[:, i * NH:(i + 1) * NH],
                            qfm[:, i * NTOK:(i + 1) * NTOK],
                            start=True, stop=True, skip_group_check=True,
                            tile_position=(0, 32 * g))
                    kfm_g.append(kfm)
                    qfm_g.append(qfm)
                    vsb_g.append(vsb)

                # ========== supergroup: Z ~= 1/ZD (fast recip, bf16 out) =====
                zpk = za.tile([HID, QF], BF16, tag="zpk")
                nc.vector._custom_dve(
                    RECIPROCAL_APPROX_FAST, out=zpk[:], in0=pzd[:],
                    s0=rc["s0"], s1=rc["s1"], imm2=rc["imm2"])

                # ================= back half =================
                for g in range(NGRP_SUPER):
                    qfm = qfm_g[g]
                    kfm = kfm_g[g]
                    rowg = slice(32 * g, 32 * g + NH)

                    # Zexp (128, 480): one expander matmul per group
                    pze = pep.tile([HID, 512], F32, tag="pze")
                    pze = pze[:, :QF]
                    nc.tensor.matmul(
                        pze[:], e8_t[rowg, :], zpk[rowg, :],
                        start=True, stop=True, tile_position=(32 * g, 0))

                    # Qbar[hd, i] = sum_l qfm * zexp : product + segmented sum
                    prodq = work.tile([HID, QF], BF16, tag="prodq")
                    qbar = small.tile([HID, GRP], F32, tag="qbar")
                    nc.vector.tensor_tensor(prodq[:], qfm[:], pze[:], ALU.mult)
                    nc.vector.tensor_reduce(
                        qbar[:], prodq[:].rearrange("p (g l) -> p g l", l=NTOK),
                        mybir.AxisListType.X, ALU.add)

                    # Abd = maskn * Qbar ; A^T rows packed at base 32*g
                    abd = small.tile([HID, GRP * NH], BF16, tag="abd")
                    nc.gpsimd.tensor_tensor(
                        abd[:].rearrange("p (g h) -> p g h", h=NH),
                        mn_t[:].rearrange("p (g h) -> p g h", h=NH),
                        qbar[:, :, None].to_broadcast((HID, GRP, NH)),
                        ALU.mult)
                    for i in range(GRP):
                        nc.tensor.matmul(
                            pa[rowg, i * S:(i + 1) * S],
                            abd[:, i * NH:(i + 1) * NH],
                            kfm[:, i * S:(i + 1) * S],
                            start=True, stop=True, skip_group_check=True,
                            tile_position=(0, 32 * g))

                # A^T -> SBUF bf16 once per supergroup
                apk = za.tile([HID, KF], BF16, tag="apk")
                nc.scalar.activation(apk[:], pa[:], AF.Copy)

                for g in range(NGRP_SUPER):
                    vsb = vsb_g[g]
                    rowg = slice(32 * g, 32 * g + NH)
                    pae = pep.tile([HID, 512], F32, tag="pze")
                    pae = pae[:, :KF]
                    nc.tensor.matmul(
                        pae[:], e8_t[rowg, :], apk[rowg, :],
                        start=True, stop=True, tile_position=(32 * g, 0))
                    # out^T[:, n] = sum_s vsb * aexp : product + segmented sum
                    prodv = work.tile([HID, KF], BF16, tag="prodv")
                    nc.vector.tensor_tensor(prodv[:], vsb[:], pae[:], ALU.mult)
                    ocol = sg * SUPER + g * GRP
                    nc.vector.tensor_reduce(
                        outT[:, ocol:ocol + GRP],
                        prodv[:].rearrange("p (g s) -> p g s", s=S),
                        mybir.AxisListType.X, ALU.add)

            nc.sync.dma_start(out[:], outT[:])

    nc.finalize()
    return nc


# ---------------- host-side packing ----------------

def make_consts():
    hd = np.arange(HID)
    e8 = (hd[None, :] // DH == (np.arange(HID) % NH)[:, None]).astype(np.float32)
    m1 = np.zeros((HID, GRP * NH), np.float32)
    for i in range(GRP):
        for h in range(NH):
            m1[h * DH:(h + 1) * DH, i * NH + h] = 1.0
    mn = (m1 / float(NTOK)).astype(np.float32)
    return e8, m1, mn


def shard_inputs(query, x, guidance, Wq, bq, Wk, bk, Wv, bv, n_loc=NLOC,
                 ncores=NCORES):
    qin = np.ascontiguousarray(
        query.transpose(0, 2, 3, 1, 4)).reshape(NTOTAL, NTOK, C)
    e8, m1, mn = make_consts()
    bf = ml_dtypes.bfloat16
    f8 = ml_dtypes.float8_e4m3
    wqr = Wq.reshape(2, HID, HID)
    wkr = Wk.reshape(2, HID, HID)
    # DoubleRow stationaries [Ki, Ko, M] flattened to [Ki, Ko*M]
    wq_dr = np.stack([wqr[0], wqr[1]], axis=1).reshape(HID, 2 * HID)
    c8 = wq_dr.astype(f8)
    cb = np.concatenate([wkr[0], wkr[1], Wv, e8, m1, mn], axis=1).astype(bf)
    fb = np.stack(
        [bq, bk, bv, np.zeros(HID, np.float32)], axis=1).astype(np.float32)
    fb_as_bf = np.ascontiguousarray(fb).view(bf)
    cb = np.concatenate([cb, fb_as_bf], axis=1)
    shared = dict(cb=cb, c8=c8)
    in_maps = []
    for i in range(ncores):
        sl = slice(i * n_loc, (i + 1) * n_loc)
        qc = qin[sl].reshape(n_loc * NTOK, C)
        xc = x[sl].reshape(n_loc * S, HID)
        gc = guidance[sl].reshape(n_loc * S, HID)
        m = dict(shared)
        m["qt"] = np.ascontiguousarray(qc.T).reshape(2, HID, n_loc * NTOK).astype(f8)
        m["xt"] = np.ascontiguousarray(xc.T).astype(bf)
        m["gt"] = np.ascontiguousarray(gc.T).astype(bf)
        in_maps.append(m)
    return in_maps


_NC_CACHE = {}


def kernel(**inputs) -> np.ndarray:
    inputs = {k: np.asarray(v, dtype=np.float32) if np.asarray(v).dtype != np.int32
              else np.asarray(v) for k, v in inputs.items()}
    in_maps = shard_inputs(**inputs)
    if NLOC not in _NC_CACHE:
        _NC_CACHE[NLOC] = build_nc(NLOC)
    nc = _NC_CACHE[NLOC]
    res = run_bass_kernel_spmd(nc, in_maps, core_ids=list(range(NCORES)))
    outs = [np.asarray(res.results[i]["o"]).T for i in range(NCORES)]
    full = np.concatenate(outs, axis=0)  # (2304, 128)
    return full.reshape(B, H_, W_, HID).astype(np.float32)


# revision 14
# speedup vs baseline: 1.2132x; 1.0506x over previous
"""Trainium2 Bass kernel for nn_AttentionLayer (linear attention, sparse_attention).

Math (per batch element n of B*H*W=2304):
    q = qin @ Wq + bq                (N=80 tokens, C=256 -> 128)
    k = [x|guidance] @ Wk + bk       (S=77 tokens)
    v = x @ Wv + bv
    Q = elu(q)+1, K = elu(k)+1       (8 heads x 16 dim)
    ZD[l,h]   = sum_d Q[l,hd]*Ksum[hd],  Ksum = sum_s K
    Z         = 1/(ZD+eps)
    Qbar[hd]  = sum_l Q[l,hd]*Z[l,h]
    A[h,s]    = sum_d Qbar[hd]*K[s,hd]
    out[hv]   = (1/N) * sum_s A[h,s]*v[s,hv]     (exact refactor of reference)

Feature layout: feature dim (128) on SBUF partitions, tokens on the free axis.
Host supplies pre-transposed inputs; output is transposed back on host.

v3: fp8(e4m3) inputs + DoubleRow projections. q/k projections contract all
256 input channels in ONE matmul (stationary holds 2 fp8 weights per PE
cell), halving projection PE time and halving input DMA. The PE runs cold
(1.2 GHz) because this dataflow can't sustain a full HAM activity window, so
fewer columns/instructions matter double.

Engine split:
  ACT    : exp+relu of q/k (bias fused), v copy (+bias), A^T copy
  GPSIMD : fm adds (min(exp,1)+relu), kbd/abd mask*broadcast products
  DVE    : min(exp,1), Ksum reduce, Z = approx-recip of ZD (eps dropped:
           ZD >> 1e-6 always), products against expander outputs +
           segmented tensor_reduce
  PE     : projections (fp8), ZD/A small matmuls, Z/A expanders (bf16)
"""

import numpy as np
import ml_dtypes

import concourse.bass as bass
import concourse.bacc as bacc
import concourse.mybir as mybir
import concourse.tile as tile
from concourse.bass_utils import run_bass_kernel_spmd
from concourse.dve_ops import RECIP_APPROX_FAST_CONSTS, RECIPROCAL_APPROX_FAST

F32 = mybir.dt.float32
BF16 = mybir.dt.bfloat16
FP8 = mybir.dt.float8e4
AF = mybir.ActivationFunctionType
ALU = mybir.AluOpType
DR = mybir.MatmulPerfMode.DoubleRow


NCORES = 8
NH, DH, HID = 8, 16, 128
S, NTOK, C = 77, 80, 256
B, N_, H_, W_ = 4, 80, 24, 24
NTOTAL = B * H_ * W_            # 2304
NLOC = NTOTAL // NCORES         # 288
GRP = 6                         # batch elements per group
NGRP_SUPER = 4                  # groups per supergroup (4 x 8 rows in PSUM bands)
SUPER = GRP * NGRP_SUPER        # 24 n per supergroup


def build_nc(n_loc: int = NLOC) -> bass.Bass:
    assert n_loc % SUPER == 0
    nsuper = n_loc // SUPER

    nc = bacc.Bacc()

    qt = nc.declare_dram_parameter("qt", [2, HID, n_loc * NTOK], FP8, isOutput=False)
    xt = nc.declare_dram_parameter("xt", [HID, n_loc * S], BF16, isOutput=False)
    gt = nc.declare_dram_parameter("gt", [HID, n_loc * S], BF16, isOutput=False)
    c8p = nc.declare_dram_parameter("c8", [HID, 2 * HID], FP8, isOutput=False)
    CBW = 4 * HID + 2 * GRP * NH + 8   # wk0|wk1|wv|e8|m1|mn|f32bits
    cbp = nc.declare_dram_parameter("cb", [HID, CBW], BF16, isOutput=False)
    out = nc.declare_dram_parameter("o", [HID, n_loc], F32, isOutput=True)

    QF = GRP * NTOK   # 480 free elems per group (q side)
    KF = GRP * S      # 462 free elems per group (k/v side)
    QS = SUPER * NTOK  # 1920 per supergroup
    KS = SUPER * S     # 1848

    rc = RECIP_APPROX_FAST_CONSTS

    with tile.TileContext(nc) as tc:
        with (
            tc.tile_pool(name="consts", bufs=1) as consts,
            tc.tile_pool(name="dmain", bufs=2) as dmain,
            tc.tile_pool(name="work", bufs=4) as work,
            tc.tile_pool(name="persist", bufs=9) as persist,
            tc.tile_pool(name="za", bufs=3) as za,
            tc.tile_pool(name="small", bufs=6) as small,
            tc.tile_pool(name="outp", bufs=1) as outp,
            tc.tile_pool(name="pq", bufs=1, space="PSUM") as pqp,
            tc.tile_pool(name="pk", bufs=1, space="PSUM") as pkp,
            tc.tile_pool(name="pv", bufs=1, space="PSUM") as pvp,
            tc.tile_pool(name="pzd", bufs=1, space="PSUM") as pzdp,
            tc.tile_pool(name="pe", bufs=3, space="PSUM") as pep,
            tc.tile_pool(name="pa", bufs=1, space="PSUM") as pap,
        ):
            # ---- constants: two packed blobs, two DMAs ----
            cb_t = consts.tile([HID, CBW], BF16)
            nc.sync.dma_start(cb_t[:], cbp[:])
            c8_t = consts.tile([HID, 2 * HID], FP8)
            nc.sync.dma_start(c8_t[:], c8p[:])
            # DoubleRow stationary: [Ki=128, Ko=2, M=128]
            wq_dr = c8_t[:, 0:2 * HID].rearrange("p (ko m) -> p ko m", m=HID)
            wk0_t = cb_t[:, 0:HID]
            wk1_t = cb_t[:, HID:2 * HID]
            wv_t = cb_t[:, 2 * HID:3 * HID]
            e8_t = cb_t[:, 3 * HID:4 * HID]
            m1_t = cb_t[:, 4 * HID:4 * HID + GRP * NH]
            mn_t = cb_t[:, 4 * HID + GRP * NH:4 * HID + 2 * GRP * NH]
            fb_t = cb_t[:, CBW - 8:CBW].bitcast(F32)
            bq_t = fb_t[:, 0:1]
            bk_t = fb_t[:, 1:2]
            bv_t = fb_t[:, 2:3]
            tc.strict_bb_all_engine_barrier()

            # HAM warm-up: chained garbage matmuls keep the PE busy >3.4us so
            # the clock gate opens before real work; overlaps the first input
            # DMA. (Steady state re-throttles regardless; this helps the head.)
            heat = pep.tile([HID, 512], F32, tag="pze")
            for h in range(14):
                nc.tensor.matmul(heat[:], cb_t[:, 0:HID], cb_t[:, 0:512],
                                 start=(h == 0), stop=(h == 13))

            outT = outp.tile([HID, n_loc], F32)

            for sg in range(nsuper):
                # ---- supergroup DMA in (fp8) ----
                qt_sb = dmain.tile([HID, 2, QS], FP8, tag="qt")
                kg_sb = dmain.tile([HID, 2, KS], BF16, tag="kg")
                for po in range(2):
                    nc.sync.dma_start(
                        qt_sb[:, po, :], qt[po, :, sg * QS:(sg + 1) * QS]
                    )
                nc.sync.dma_start(kg_sb[:, 0, :], xt[:, sg * KS:(sg + 1) * KS])
                nc.sync.dma_start(kg_sb[:, 1, :], gt[:, sg * KS:(sg + 1) * KS])

                # supergroup-lifetime PSUM bands (4 groups x 8 rows each);
                # rows 32g+8..32g+32 are never written and never read.
                pzd = pzdp.tile([HID, 512], F32, tag="pzd")
                pzd = pzd[:, :QF]
                pa = pap.tile([HID, 512], F32, tag="pa")
                pa = pa[:, :KF]

                kfm_g = []
                qfm_g = []
                vsb_g = []
                # ================= front half: proj + fm + ZD =================
                for g in range(NGRP_SUPER):
                    qs = slice(g * QF, (g + 1) * QF)
                    ks = slice(g * KF, (g + 1) * KF)

                    pq = pqp.tile([HID, 512], F32, tag="pq")
                    pq = pq[:, :QF]
                    pk = pkp.tile([HID, 512], F32, tag="pk")
                    pk = pk[:, :KF]
                    pv = pvp.tile([HID, 512], F32, tag="pv")
                    pv = pv[:, :KF]
                    nc.tensor.matmul(pq[:], wq_dr, qt_sb[:, :, qs],
                                     start=True, stop=True, perf_mode=DR)
                    nc.tensor.matmul(pk[:], wk0_t, kg_sb[:, 0, ks],
                                     start=True, stop=False)
                    nc.tensor.matmul(pk[:], wk1_t, kg_sb[:, 1, ks],
                                     start=False, stop=True)
                    nc.tensor.matmul(pv[:], wv_t, kg_sb[:, 0, ks],
                                     start=True, stop=True)

                    # feature map: fm(y) = min(exp(y), 1) + relu(y)
                    eq = work.tile([HID, QF], BF16, tag="eq")
                    rq = work.tile([HID, QF], BF16, tag="rq")
                    qfm = persist.tile([HID, QF], BF16, tag="qfm")
                    ek = work.tile([HID, KF], BF16, tag="ek")
                    rk = work.tile([HID, KF], BF16, tag="rk")
                    kfm = persist.tile([HID, KF], BF16, tag="kfm")
                    vsb = persist.tile([HID, KF], BF16, tag="vsb")
                    nc.scalar.activation(eq[:], pq[:], AF.Exp, bias=bq_t)
                    nc.scalar.activation(rq[:], pq[:], AF.Relu, bias=bq_t)
                    nc.scalar.activation(ek[:], pk[:], AF.Exp, bias=bk_t)
                    nc.scalar.activation(rk[:], pk[:], AF.Relu, bias=bk_t)
                    nc.scalar.activation(vsb[:], pv[:], AF.Identity, bias=bv_t)
                    eqm = work.tile([HID, QF], BF16, tag="eqm")
                    ekm = work.tile([HID, KF], BF16, tag="ekm")
                    nc.vector.tensor_scalar_min(eqm[:], eq[:], 1.0)
                    nc.vector.tensor_scalar_min(ekm[:], ek[:], 1.0)
                    nc.gpsimd.tensor_tensor(qfm[:], eqm[:], rq[:], ALU.add)
                    nc.gpsimd.tensor_tensor(kfm[:], ekm[:], rk[:], ALU.add)

                    # Ksum (128, GRP) then KBD = mask1 * Ksum  (8 cols per n)
                    ksum = small.tile([HID, GRP], F32, tag="ksum")
                    kbd = small.tile([HID, GRP * NH], BF16, tag="kbd")
                    nc.vector.tensor_reduce(
                        ksum[:], kfm[:].rearrange("p (g s) -> p g s", s=S),
                        mybir.AxisListType.X, ALU.add)
                    nc.gpsimd.tensor_tensor(
                        kbd[:].rearrange("p (g h) -> p g h", h=NH),
                        m1_t[:].rearrange("p (g h) -> p g h", h=NH),
                        ksum[:, :, None].to_broadcast((HID, GRP, NH)),
                        ALU.mult)

                    # ZD rows for this group -> packed at partition base 32*g
                    for i in range(GRP):
                        nc.tensor.matmul(
                            pzd[32 * g:32 * g + NH, i * NTOK:(i + 1) * NTOK],
                            kbd[:, i * NH:(i + 1) * NH],
                            qfm[:, i * NTOK:(i + 1) * NTOK],
                            start=True, stop=True, skip_group_check=True,
                            tile_position=(0, 32 * g))
                    kfm_g.append(kfm)
                    qfm_g.append(qfm)
                    vsb_g.append(vsb)

                # ========== supergroup: Z ~= 1/ZD (fast recip, bf16 out) =====
                zpk = za.tile([HID, QF], BF16, tag="zpk")
                nc.vector._custom_dve(
                    RECIPROCAL_APPROX_FAST, out=zpk[:], in0=pzd[:],
                    s0=rc["s0"], s1=rc["s1"], imm2=rc["imm2"])

                # ================= back half =================
                for g in range(NGRP_SUPER):
                    qfm = qfm_g[g]
                    kfm = kfm_g[g]
                    rowg = slice(32 * g, 32 * g + NH)

                    # Zexp (128, 480): one expander matmul per group
                    pze = pep.tile([HID, 512], F32, tag="pze")
                    pze = pze[:, :QF]
                    nc.tensor.matmul(
                        pze[:], e8_t[rowg, :], zpk[rowg, :],
                        start=True, stop=True, tile_position=(32 * g, 0))

                    # Qbar[hd, i] = sum_l qfm * zexp : product + segmented sum
                    prodq = work.tile([HID, QF], BF16, tag="prodq")
                    qbar = small.tile([HID, GRP], F32, tag="qbar")
                    nc.vector.tensor_tensor(prodq[:], qfm[:], pze[:], ALU.mult)
                    nc.vector.tensor_reduce(
                        qbar[:], prodq[:].rearrange("p (g l) -> p g l", l=NTOK),
                        mybir.AxisListType.X, ALU.add)

                    # Abd = maskn * Qbar ; A^T rows packed at base 32*g
                    abd = small.tile([HID, GRP * NH], BF16, tag="abd")
                    nc.gpsimd.tensor_tensor(
                        abd[:].rearrange("p (g h) -> p g h", h=NH),
                        mn_t[:].rearrange("p (g h) -> p g h", h=NH),
                        qbar[:, :, None].to_broadcast((HID, GRP, NH)),
                        ALU.mult)
                    for i in range(GRP):
                        nc.tensor.matmul(
                            pa[rowg, i * S:(i + 1) * S],
                            abd[:, i * NH:(i + 1) * NH],
                            kfm[:, i * S:(i + 1) * S],
                            start=True, stop=True, skip_group_check=True,
                            tile_position=(0, 32 * g))

                # A^T -> SBUF bf16 once per supergroup
                apk = za.tile([HID, KF], BF16, tag="apk")
                nc.scalar.activation(apk[:], pa[:], AF.Copy)

                for g in range(NGRP_SUPER):
                    vsb = vsb_g[g]
                    rowg = slice(32 * g, 32 * g + NH)
                    pae = pep.tile([HID, 512], F32, tag="pze")
                    pae = pae[:, :KF]
                    nc.tensor.matmul(
                        pae[:], e8_t[rowg, :], apk[rowg, :],
                        start=True, stop=True, tile_position=(32 * g, 0))
                    # out^T[:, n] = sum_s vsb * aexp : product + segmented sum
                    prodv = work.tile([HID, KF], BF16, tag="prodv")
                    nc.vector.tensor_tensor(prodv[:], vsb[:], pae[:], ALU.mult)
                    ocol = sg * SUPER + g * GRP
                    nc.vector.tensor_reduce(
                        outT[:, ocol:ocol + GRP],
                        prodv[:].rearrange("p (g s) -> p g s", s=S),
                        mybir.AxisListType.X, ALU.add)

            nc.sync.dma_start(out[:], outT[:])

    nc.finalize()
    return nc


# ---------------- host-side packing ----------------

def make_consts():
    hd = np.arange(HID)
    e8 = (hd[None, :] // DH == (np.arange(HID) % NH)[:, None]).astype(np.float32)
    m1 = np.zeros((HID, GRP * NH), np.float32)
    for i in range(GRP):
        for h in range(NH):
            m1[h * DH:(h + 1) * DH, i * NH + h] = 1.0
    mn = (m1 / float(NTOK)).astype(np.float32)
    return e8, m1, mn


def shard_inputs(query, x, guidance, Wq, bq, Wk, bk, Wv, bv, n_loc=NLOC,
                 ncores=NCORES):
    qin = np.ascontiguousarray(
        query.transpose(0, 2, 3, 1, 4)).reshape(NTOTAL, NTOK, C)
    e8, m1, mn = make_consts()
    bf = ml_dtypes.bfloat16
    f8 = ml_dtypes.float8_e4m3
    wqr = Wq.reshape(2, HID, HID)
    wkr = Wk.reshape(2, HID, HID)
    # DoubleRow stationaries [Ki, Ko, M] flattened to [Ki, Ko*M]
    wq_dr = np.stack([wqr[0], wqr[1]], axis=1).reshape(HID, 2 * HID)
    c8 = wq_dr.astype(f8)
    cb = np.concatenate([wkr[0], wkr[1], Wv, e8, m1, mn], axis=1).astype(bf)
    fb = np.stack(
        [bq, bk, bv, np.zeros(HID, np.float32)], axis=1).astype(np.float32)
    fb_as_bf = np.ascontiguousarray(fb).view(bf)
    cb = np.concatenate([cb, fb_as_bf], axis=1)
    shared = dict(cb=cb, c8=c8)
    in_maps = []
    for i in range(ncores):
        sl = slice(i * n_loc, (i + 1) * n_loc)
        qc = qin[sl].reshape(n_loc * NTOK, C)
        xc = x[sl].reshape(n_loc * S, HID)
        gc = guidance[sl].reshape(n_loc * S, HID)
        m = dict(shared)
        m["qt"] = np.ascontiguousarray(qc.T).reshape(2, HID, n_loc * NTOK).astype(f8)
        m["xt"] = np.ascontiguousarray(xc.T).astype(bf)
        m["gt"] = np.ascontiguousarray(gc.T).astype(bf)
        in_maps.append(m)
    return in_maps


_NC_CACHE = {}


def kernel(**inputs) -> np.ndarray:
    inputs = {k: np.asarray(v, dtype=np.float32) if np.asarray(v).dtype != np.int32
              else np.asarray(v) for k, v in inputs.items()}
    in_maps = shard_inputs(**inputs)
    if NLOC not in _NC_CACHE:
        _NC_CACHE[NLOC] = build_nc(NLOC)
    nc = _NC_CACHE[NLOC]
    res = run_bass_kernel_spmd(nc, in_maps, core_ids=list(range(NCORES)))
    outs = [np.asarray(res.results[i]["o"]).T for i in range(NCORES)]
    full = np.concatenate(outs, axis=0)  # (2304, 128)
    return full.reshape(B, H_, W_, HID).astype(np.float32)
